# revision 2
# baseline (speedup 1.0000x reference)
"""Fused single-head attention kernel for 8 TRN2 NeuronCores.

Problem: B=4, S=2048, D=1024 attention:
    Q = x @ Wq.T + bq; K = x @ Wk.T + bk; V = x @ Wv.T + bv
    out = softmax(Q K^T / sqrt(D)) @ V

Sharding (no cross-core comms): core c handles batch b = c//2 and query
half h = c%2 (1024 queries). Each core computes K/V for its full batch
(duplicated across the pair) and Q only for its query half.

Host-side preprocessing per core:
  - x[b] is transposed to xT [d, s] (the contraction dim d must sit on
    SBUF partitions) and ROTATED along s by -h*1024 so that this core's
    query half is always columns 0..1023 of xT. Attention is
    permutation-invariant along the key axis, so rotating K/V rows
    consistently does not change the output.
  - Wq/Wk/Wv are transposed to WT [d, e]; Wq and bq are pre-scaled by
    1/sqrt(D) so no scaling is needed on device.
  - bv is folded in AFTER normalization (softmax rows sum to 1, so
    out = (P @ (x WvT))/rowsum(P) + bv), pre-broadcast to [128, e] on
    host so the device does a plain elementwise add.

Device (per core), all matmul inputs bf16, accumulation fp32:
  phase A: QT[e,q] = (WqT)^T.T @ xT, KT[e,s], V[s,e] projections
  phase B: for each q-block of 256:
     attT[k,q] = KT_tile^T.T @ QT  (k on partitions -> softmax sums and
     the PV matmul both come out in natural layouts with no transposes)
     PT = exp(attT) (ScalarE, PSUM->SBUF)
     S[q] += PT^T @ ones ; out[q,e] += PT^T @ V   (PSUM accumulated over k)
     out = out * (1/S) + bv  -> DMA to DRAM
"""

import os
import sys

for _p in ("/opt/trn_rl_repo", "/root/.axon_site/_ro/trn_rl_repo"):
    if os.path.isdir(_p) and _p not in sys.path:
        sys.path.insert(0, _p)

import numpy as np
import ml_dtypes

import concourse.bass as bass
import concourse.tile as tile
from concourse import bacc, mybir
from concourse.bass_utils import run_bass_kernel_spmd

BF16 = ml_dtypes.bfloat16
F32 = mybir.dt.float32
CDT = mybir.dt.bfloat16

B, S, D = 4, 2048, 1024
N_CORES = 8
P = 128
DT = D // P          # 8 d-tiles (contraction)
ET = D // P          # 8 e-tiles
KT_N = S // P        # 16 k-tiles
QH = S // 2          # 1024 queries per core
QB = 256             # q-block for phase B
NQB = QH // QB       # 4 q-blocks
QS = QB // P         # 2 q-subtiles per block

_NC_CACHE = {}


def build_nc(reps: int = 1):
    nc = bacc.Bacc("TRN2", target_bir_lowering=False, debug=False,
                   num_devices=N_CORES)
    Exp = mybir.ActivationFunctionType.Exp
    Ident = mybir.ActivationFunctionType.Identity

    xT_d = nc.dram_tensor("xT", [D, S], CDT, kind="ExternalInput").ap()
    wqT_d = nc.dram_tensor("WqT", [D, D], CDT, kind="ExternalInput").ap()
    wkT_d = nc.dram_tensor("WkT", [D, D], CDT, kind="ExternalInput").ap()
    wvT_d = nc.dram_tensor("WvT", [D, D], CDT, kind="ExternalInput").ap()
    bq_d = nc.dram_tensor("bq2", [P, DT], F32, kind="ExternalInput").ap()
    bk_d = nc.dram_tensor("bk2", [P, DT], F32, kind="ExternalInput").ap()
    bv_d = nc.dram_tensor("bvb", [P, D], F32, kind="ExternalInput").ap()
    out_d = nc.dram_tensor("out", [QH, D], F32, kind="ExternalOutput").ap()

    with tile.TileContext(nc) as tc:
        with (
            tc.tile_pool(name="resident", bufs=1) as res,
            tc.tile_pool(name="wpool", bufs=2) as wpool,
            tc.tile_pool(name="pt", bufs=3) as ptpool,
            tc.tile_pool(name="osb", bufs=4) as opool,
            tc.tile_pool(name="small", bufs=4) as spool,
            tc.tile_pool(name="ps", bufs=2, space="PSUM") as psA,
            tc.tile_pool(name="pso", bufs=4, space="PSUM") as psO,
            tc.tile_pool(name="pss", bufs=2, space="PSUM") as psS,
        ):
            # ---- resident loads (once) ----
            xt = [res.tile([P, S], CDT, tag=f"xt{d}", name=f"xt{d}") for d in range(DT)]
            for d in range(DT):
                nc.sync.dma_start(xt[d][:], xT_d[d * P:(d + 1) * P, :])
            bq_sb = res.tile([P, DT], F32, tag="bq", name="bq_sb")
            nc.sync.dma_start(bq_sb[:], bq_d[:, :])
            bk_sb = res.tile([P, DT], F32, tag="bk", name="bk_sb")
            nc.sync.dma_start(bk_sb[:], bk_d[:, :])
            bv_sb = res.tile([P, D], F32, tag="bv", name="bv_sb")
            nc.sync.dma_start(bv_sb[:], bv_d[:, :])
            ones = res.tile([P, 1], CDT, tag="ones", name="ones")
            nc.vector.memset(ones[:], 1.0)

            qt = [res.tile([P, QH], CDT, tag=f"qt{e}", name=f"qt{e}") for e in range(ET)]
            kt = [res.tile([P, S], CDT, tag=f"kt{e}", name=f"kt{e}") for e in range(ET)]
            vt = [res.tile([P, D], CDT, tag=f"vt{s}", name=f"vt{s}") for s in range(KT_N)]

            for _rep in range(reps):
                # ---- phase A: projections ----
                # QT[e,q] / KT[e,s]: lhsT = WT[d, e-tile], rhs = xT[d, s]
                for which, w_dram, bias_sb, dst, ncols in (
                    ("q", wqT_d, bq_sb, qt, QH),
                    ("k", wkT_d, bk_sb, kt, S),
                ):
                    w_sb = wpool.tile([P, DT, D], CDT, tag="w", name="w_sb")
                    nc.sync.dma_start(
                        w_sb[:],
                        w_dram.rearrange("(do dp) e -> dp do e", dp=P))
                    for e in range(ET):
                        for sb in range(ncols // 512):
                            ps = psA.tile([P, 512], F32, tag="ps", name="ps")
                            for d in range(DT):
                                nc.tensor.matmul(
                                    ps[:],
                                    lhsT=w_sb[:, d, e * P:(e + 1) * P],
                                    rhs=xt[d][:, sb * 512:(sb + 1) * 512],
                                    start=(d == 0), stop=(d == DT - 1))
                            nc.scalar.activation(
                                dst[e][:, sb * 512:(sb + 1) * 512], ps[:],
                                Ident, bias=bias_sb[:, e:e + 1])
                # V[s,e]: lhsT = xT[d, s-tile], rhs = WvT[d, e]  (no bias)
                wv_sb = wpool.tile([P, DT, D], CDT, tag="w", name="w_sb")
                nc.sync.dma_start(
                    wv_sb[:], wvT_d.rearrange("(do dp) e -> dp do e", dp=P))
                for s in range(KT_N):
                    for eb in range(D // 512):
                        ps = psA.tile([P, 512], F32, tag="ps", name="ps")
                        for d in range(DT):
                            nc.tensor.matmul(
                                ps[:],
                                lhsT=xt[d][:, s * P:(s + 1) * P],
                                rhs=wv_sb[:, d, eb * 512:(eb + 1) * 512],
                                start=(d == 0), stop=(d == DT - 1))
                        nc.vector.tensor_copy(
                            out=vt[s][:, eb * 512:(eb + 1) * 512], in_=ps[:])

                # ---- phase B: attention, q-block at a time ----
                for qb in range(NQB):
                    pso = [psO.tile([P, 512], F32, tag="pso", name="pso")
                           for _ in range(QS * 2)]
                    pss = [psS.tile([P, 1], F32, tag="pss", name="pss")
                           for _ in range(QS)]
                    for k in range(KT_N):
                        psa = psA.tile([P, QB], F32, tag="ps", name="psa")
                        for e in range(ET):
                            nc.tensor.matmul(
                                psa[:],
                                lhsT=kt[e][:, k * P:(k + 1) * P],
                                rhs=qt[e][:, qb * QB:(qb + 1) * QB],
                                start=(e == 0), stop=(e == ET - 1))
                        pt_sb = ptpool.tile([P, QB], CDT, tag="pt", name="pt_sb")
                        nc.scalar.activation(pt_sb[:], psa[:], Exp)
                        for qs in range(QS):
                            lhs = pt_sb[:, qs * P:(qs + 1) * P]
                            nc.tensor.matmul(
                                pss[qs][:], lhsT=lhs, rhs=ones[:],
                                start=(k == 0), stop=(k == KT_N - 1))
                            for eb in range(2):
                                nc.tensor.matmul(
                                    pso[qs * 2 + eb][:], lhsT=lhs,
                                    rhs=vt[k][:, eb * 512:(eb + 1) * 512],
                                    start=(k == 0), stop=(k == KT_N - 1))
                    for qs in range(QS):
                        rec = spool.tile([P, 1], F32, tag="rec", name="rec")
                        nc.vector.reciprocal(rec[:], pss[qs][:])
                        for eb in range(2):
                            osb = opool.tile([P, 512], F32, tag="osb", name="osb")
                            nc.vector.tensor_scalar(
                                osb[:], pso[qs * 2 + eb][:], rec[:], None,
                                mybir.AluOpType.mult)
                            nc.vector.tensor_add(
                                osb[:], osb[:],
                                bv_sb[:, eb * 512:(eb + 1) * 512])
                            row = qb * QB + qs * P
                            nc.sync.dma_start(
                                out_d[row:row + P, eb * 512:(eb + 1) * 512],
                                osb[:])
    nc.compile()
    return nc


def _get_nc(reps: int = 1):
    if reps not in _NC_CACHE:
        _NC_CACHE[reps] = build_nc(reps)
    return _NC_CACHE[reps]


def make_in_maps(x, Wq, bq, Wk, bk, Wv, bv):
    inv = np.float32(1.0 / np.sqrt(D))
    wqT = np.ascontiguousarray((Wq.T * inv)).astype(BF16)
    wkT = np.ascontiguousarray(Wk.T).astype(BF16)
    wvT = np.ascontiguousarray(Wv.T).astype(BF16)
    bq2 = np.ascontiguousarray((bq * inv).reshape(DT, P).T).astype(np.float32)
    bk2 = np.ascontiguousarray(bk.reshape(DT, P).T).astype(np.float32)
    bvb = np.ascontiguousarray(np.broadcast_to(bv, (P, D))).astype(np.float32)
    in_maps = []
    for c in range(N_CORES):
        b, h = divmod(c, 2)
        xT = np.ascontiguousarray(x[b].T)          # [D, S]
        xTr = np.roll(xT, -h * QH, axis=1)         # this core's queries first
        in_maps.append({
            "xT": np.ascontiguousarray(xTr).astype(BF16),
            "WqT": wqT, "WkT": wkT, "WvT": wvT,
            "bq2": bq2, "bk2": bk2, "bvb": bvb,
        })
    return in_maps


def kernel(x, Wq, bq, Wk, bk, Wv, bv):
    x = np.asarray(x, np.float32)
    in_maps = make_in_maps(x, np.asarray(Wq, np.float32),
                           np.asarray(bq, np.float32),
                           np.asarray(Wk, np.float32),
                           np.asarray(bk, np.float32),
                           np.asarray(Wv, np.float32),
                           np.asarray(bv, np.float32))
    nc = _get_nc()
    res = run_bass_kernel_spmd(nc, in_maps, core_ids=list(range(N_CORES)))
    out = np.empty((B, S, D), np.float32)
    for c in range(N_CORES):
        b, h = divmod(c, 2)
        out[b, h * QH:(h + 1) * QH, :] = res.results[c]["out"]
    return out


# revision 18
# speedup vs baseline: 3.2270x; 3.2270x over previous
"""Fused single-head attention kernel for 8 TRN2 NeuronCores.

Problem: B=4, S=2048, D=1024 attention:
    Q = x @ Wq.T + bq; K = x @ Wk.T + bk; V = x @ Wv.T + bv
    out = softmax(Q K^T / sqrt(D)) @ V

Sharding (no cross-core traffic): core c handles batch b = c//2 and
query half h = c%2 (1024 queries).

The kernel uses an algebraic refactoring that removes the K and V
projections (and with them any need to exchange K/V between the two
cores of a batch pair):

  logits = (x_q Wq^T + bq)(x_k Wk^T + bk)^T / sqrt(D)
         = x_q M2 x_k^T + x_k.z + (per-q terms), M2 = Wq^T Wk / sqrt(D)
  - the per-QUERY additive terms (x_q Wq^T bk and bq.bk) are constant
    along the softmax axis and drop out of the softmax entirely;
  - the per-KEY term x_k.(Wk^T bq)/sqrt(D) = x_k.z survives and is
    folded into Y (Y' = M2^T x_q^T + z 1^T, added as the per-partition
    bias of Y's evacuation), so it costs nothing.
  M2 [d,d] and z [d] depend only on the weights and are precomputed on
  the host (f64) - weight fusion, no runtime data involved.

  out = softmax @ (x Wv^T) + bv = ((P x) Wv^T)/rowsum(P) + bv
  so V is never materialized: first tmp = P^T.T @ x (attention-weighted
  inputs), then one [1024,1024] projection by Wv^T at the end.

Per-core device FLOPs drop from 15.0 to 12.9 GFLOP and all tensors are
core-local (x is passed in three host-prepared layouts: xT [d,s] full,
xN [s,d] full, xqT [d,q] the core's query half).

Device dataflow (all matmul inputs bf16, accumulation fp32):
  Y[dc,q] = M2^T.T @ xqT + z      (128 MMs, bias in the evac)
  per q-block of 512 (pass1/2/3):
    pass1: attT[k,q] += xT_slice^T.T @ Y  (k on partitions -> softmax
      sums and the downstream matmuls all need no transposes);
      PT = exp(attT) (ScalarE, PSUM->SBUF, kept for pass 2);
      rowsum via ones-stationary matmul -> S_row [1,512] (PSUM, 1 bank);
      1/S: evac, transpose 128-slices via K=1 matmul, DVE reciprocal
    pass2: tmpT[d,q] += xN_slice^T.T @ PT  (PSUM over 16 k-tiles)
    pass3: out[q,e] += tmpT_slice^T.T @ WvT; evac with
      out = out * (1/S) + bv -> DMA to DRAM
"""

import os
import sys

for _p in ("/opt/trn_rl_repo", "/root/.axon_site/_ro/trn_rl_repo"):
    if os.path.isdir(_p) and _p not in sys.path:
        sys.path.insert(0, _p)

import numpy as np
import ml_dtypes

import concourse.bass as bass
import concourse.tile as tile
from concourse import bacc, mybir
from concourse.bass_utils import run_bass_kernel_spmd

BF16 = ml_dtypes.bfloat16
F32 = mybir.dt.float32
CDT = mybir.dt.bfloat16

B, S, D = 4, 2048, 1024
N_CORES = 8
P = 128
DT = D // P          # 8 d-tiles (contraction)
KT_N = S // P        # 16 k-tiles
QH = S // 2          # 1024 queries per core
QB = 512             # q-block for phase B
NQB = QH // QB       # 2 q-blocks
QS = QB // P         # 4 q-subtiles per block

_NC_CACHE = {}


def build_nc(reps: int = 1, mode: str = "full"):
    nc = bacc.Bacc("TRN2", target_bir_lowering=False, debug=False,
                   num_devices=N_CORES)
    Exp = mybir.ActivationFunctionType.Exp
    Copy = mybir.ActivationFunctionType.Copy

    xT_d = nc.dram_tensor("xT", [D, S], CDT, kind="ExternalInput").ap()
    xN_d = nc.dram_tensor("xN", [S, D], CDT, kind="ExternalInput").ap()
    xq_d = nc.dram_tensor("xqT", [D, QH], CDT, kind="ExternalInput").ap()
    m2_d = nc.dram_tensor("M2", [D, D], CDT, kind="ExternalInput").ap()
    wvT_d = nc.dram_tensor("WvT", [D, D], CDT, kind="ExternalInput").ap()
    z_d = nc.dram_tensor("z2", [P, DT], F32, kind="ExternalInput").ap()
    bv_d = nc.dram_tensor("bvb", [P, D], F32, kind="ExternalInput").ap()
    out_d = nc.dram_tensor("out", [QH, D], F32, kind="ExternalOutput").ap()

    with tile.TileContext(nc) as tc:
        with (
            tc.tile_pool(name="resident", bufs=1) as res,
            tc.tile_pool(name="wpool", bufs=2) as wpool,
            tc.tile_pool(name="pt", bufs=2) as ptpool,
            tc.tile_pool(name="tm", bufs=2) as tmpool,
            tc.tile_pool(name="osb", bufs=4) as opool,
            tc.tile_pool(name="small", bufs=4) as spool,
            tc.tile_pool(name="ps", bufs=2, space="PSUM") as psA,
            tc.tile_pool(name="ptm", bufs=4, space="PSUM") as psT,
            tc.tile_pool(name="pss", bufs=2, space="PSUM") as psS,
        ):
            # ---- resident loads (once) ----
            # order matters for the single-shot prologue: the Y matmuls
            # need z + M2 + xqT first; xT (pass1) next; xN (pass2) and
            # bv (epilogue) last. xT/xN ride the scalar HWDGE queue so
            # they stream in parallel with the sync-queue loads.
            z_sb = res.tile([P, DT], F32, tag="z", name="z_sb")
            nc.scalar.dma_start(z_sb[:], z_d[:, :])
            m2 = [wpool.tile([P, D], CDT, tag=f"w{d}", name=f"m2_{d}")
                  for d in range(DT)]
            for d in range(DT):
                nc.sync.dma_start(m2[d][:], m2_d[d * P:(d + 1) * P, :])
            xq = [res.tile([P, QH], CDT, tag=f"xq{d}", name=f"xq{d}")
                  for d in range(DT)]
            for d in range(DT):
                nc.scalar.dma_start(xq[d][:], xq_d[d * P:(d + 1) * P, :])
            xt = [res.tile([P, S], CDT, tag=f"xt{d}", name=f"xt{d}")
                  for d in range(DT)]
            for d in range(DT):
                (nc.sync if d % 2 else nc.scalar).dma_start(
                    xt[d][:], xT_d[d * P:(d + 1) * P, :])
            xn = [res.tile([P, D], CDT, tag=f"xn{k}", name=f"xn{k}")
                  for k in range(KT_N)]
            for k in range(KT_N):
                (nc.sync if k % 2 else nc.scalar).dma_start(
                    xn[k][:], xN_d[k * P:(k + 1) * P, :])
            bv_sb = res.tile([P, D], F32, tag="bv", name="bv_sb")
            nc.scalar.dma_start(bv_sb[:], bv_d[:, :])
            ones = res.tile([P, 1], CDT, tag="ones", name="ones")
            nc.vector.memset(ones[:], 1.0)
            one11 = res.tile([1, 1], F32, tag="one11", name="one11")
            nc.vector.memset(one11[:], 1.0)

            yt = [res.tile([P, QH], CDT, tag=f"yt{d}", name=f"yt{d}")
                  for d in range(DT)]

            a_iters = range(reps) if mode in ("full", "A") else range(1)
            b_iters = range(reps) if mode in ("full", "B") else range(1)
            wv = None
            for _i_rep, _rep in enumerate(a_iters):
                # ---- Y = M2^T.T @ xqT : [dc, q] ----
                if _i_rep > 0:
                    m2 = [wpool.tile([P, D], CDT, tag=f"w{d}",
                                     name=f"m2_{d}") for d in range(DT)]
                    for d in range(DT):
                        nc.sync.dma_start(m2[d][:],
                                          m2_d[d * P:(d + 1) * P, :])
                for dc in range(DT):
                    for sb in range(QH // 512):
                        ps = psA.tile([P, 512], F32, tag="ps", name="ps")
                        for d in range(DT):
                            nc.tensor.matmul(
                                ps[:],
                                lhsT=m2[d][:, dc * P:(dc + 1) * P],
                                rhs=xq[d][:, sb * 512:(sb + 1) * 512],
                                start=(d == 0), stop=(d == DT - 1))
                        nc.scalar.activation(
                            yt[dc][:, sb * 512:(sb + 1) * 512], ps[:],
                            mybir.ActivationFunctionType.Identity,
                            bias=z_sb[:, dc:dc + 1])
                # WvT loads reuse the w{d} slots once M2 is consumed
                wv = [wpool.tile([P, D], CDT, tag=f"w{d}", name=f"wv_{d}")
                      for d in range(DT)]
                for d in range(DT):
                    nc.sync.dma_start(wv[d][:], wvT_d[d * P:(d + 1) * P, :])

            for _rep in b_iters:
                for qb in range(NQB):
                    # ---- pass 1: scores, exp(+v3 bias), row sums ----
                    srow_ps = psS.tile([1, QB], F32, tag="pss", name="srow_ps")
                    pts = []
                    for k in range(KT_N):
                        psa = psA.tile([P, QB], F32, tag="ps", name="psa")
                        for d in range(DT):
                            nc.tensor.matmul(
                                psa[:],
                                lhsT=xt[d][:, k * P:(k + 1) * P],
                                rhs=yt[d][:, qb * QB:(qb + 1) * QB],
                                start=(d == 0), stop=(d == DT - 1))
                        pt_sb = ptpool.tile([P, QB], CDT, tag=f"pt{k}",
                                            name=f"pt_sb{k}")
                        nc.scalar.activation(pt_sb[:], psa[:], Exp)
                        pts.append(pt_sb)
                        nc.tensor.matmul(
                            srow_ps[:], lhsT=ones[:], rhs=pt_sb[:],
                            start=(k == 0), stop=(k == KT_N - 1))
                    srow_sb = spool.tile([1, QB], F32, tag="srow",
                                         name="srow_sb")
                    nc.scalar.copy(srow_sb[:], srow_ps[:])
                    recs = []
                    for qs in range(QS):
                        scol_ps = psA.tile([P, 1], F32, tag="ps",
                                           name="scol_ps")
                        nc.tensor.matmul(
                            scol_ps[:],
                            lhsT=srow_sb[0:1, qs * P:(qs + 1) * P],
                            rhs=one11[:], start=True, stop=True)
                        rec = spool.tile([P, 1], F32, tag="rec", name="rec")
                        nc.vector.reciprocal(rec[:], scol_ps[:])
                        recs.append(rec)
                    # ---- pass 2: tmpT[d, q] = sum_k x_k^T P^T ----
                    tms = []
                    for dt_i in range(DT):
                        pst = psT.tile([P, 512], F32, tag="ptm", name="pst")
                        for k in range(KT_N):
                            nc.tensor.matmul(
                                pst[:],
                                lhsT=xn[k][:, dt_i * P:(dt_i + 1) * P],
                                rhs=pts[k][:],
                                start=(k == 0), stop=(k == KT_N - 1))
                        tm = tmpool.tile([P, 512], CDT, tag=f"tm{dt_i}",
                                         name=f"tm{dt_i}")
                        nc.vector.tensor_copy(out=tm[:], in_=pst[:])
                        tms.append(tm)
                    # ---- pass 3: out[q, e] = tmpT^T @ WvT, scaled + bv ----
                    for qs in range(QS):
                        for eb in range(2):
                            pso = psA.tile([P, 512], F32, tag="ps",
                                           name="pso")
                            for dt_i in range(DT):
                                nc.tensor.matmul(
                                    pso[:],
                                    lhsT=tms[dt_i][:, qs * P:(qs + 1) * P],
                                    rhs=wv[dt_i][:, eb * 512:(eb + 1) * 512],
                                    start=(dt_i == 0), stop=(dt_i == DT - 1))
                            osb = opool.tile([P, 512], F32, tag="osb",
                                             name="osb")
                            nc.scalar.activation(osb[:], pso[:], Copy,
                                                 scale=recs[qs][:])
                            nc.vector.tensor_add(
                                osb[:], osb[:],
                                bv_sb[:, eb * 512:(eb + 1) * 512])
                            row = qb * QB + qs * P
                            nc.sync.dma_start(
                                out_d[row:row + P, eb * 512:(eb + 1) * 512],
                                osb[:])
            if mode == "A":
                nc.gpsimd.dma_start(out_d[0:P, 0:8], yt[0][:, 0:8])
    nc.compile()
    return nc


def _get_nc(reps: int = 1, mode: str = "full"):
    key = (reps, mode)
    if key not in _NC_CACHE:
        _NC_CACHE[key] = build_nc(reps, mode)
    return _NC_CACHE[key]


def make_in_maps(x, Wq, bq, Wk, bk, Wv, bv):
    inv = np.float64(1.0 / np.sqrt(D))
    M2 = Wq.T.astype(np.float64) @ Wk.astype(np.float64) * inv
    z = Wk.T.astype(np.float64) @ bq.astype(np.float64) * inv
    m2b = np.ascontiguousarray(M2.astype(np.float32)).astype(BF16)
    wvT = np.ascontiguousarray(Wv.T).astype(BF16)
    z2 = np.ascontiguousarray(
        z.astype(np.float32).reshape(DT, P).T).astype(np.float32)
    bvb = np.ascontiguousarray(np.broadcast_to(bv, (P, D))).astype(np.float32)
    in_maps = []
    for c in range(N_CORES):
        b, h = divmod(c, 2)
        xT = np.ascontiguousarray(x[b].T).astype(BF16)
        xN = np.ascontiguousarray(x[b]).astype(BF16)
        xqT = np.ascontiguousarray(xT[:, h * QH:(h + 1) * QH])
        in_maps.append({
            "xT": xT, "xN": xN, "xqT": xqT,
            "M2": m2b, "WvT": wvT,
            "z2": z2, "bvb": bvb,
        })
    return in_maps


def kernel(x, Wq, bq, Wk, bk, Wv, bv):
    x = np.asarray(x, np.float32)
    in_maps = make_in_maps(x, np.asarray(Wq, np.float32),
                           np.asarray(bq, np.float32),
                           np.asarray(Wk, np.float32),
                           np.asarray(bk, np.float32),
                           np.asarray(Wv, np.float32),
                           np.asarray(bv, np.float32))
    nc = _get_nc()
    res = run_bass_kernel_spmd(nc, in_maps, core_ids=list(range(N_CORES)))
    out = np.empty((B, S, D), np.float32)
    for c in range(N_CORES):
        b, h = divmod(c, 2)
        out[b, h * QH:(h + 1) * QH, :] = res.results[c]["out"]
    return out
